# revision 4
# baseline (speedup 1.0000x reference)
"""MoE layer (top-2 of 8 experts) on 8 Trainium2 NeuronCores.

Strategy (expert parallelism, host-side dispatch):
  - Router (x @ Wr -> softmax -> top-k) is computed on host: it is ~0.05% of
    the total FLOPs.  Decisions use float64 so near-ties resolve exactly.
  - Tokens are gathered per expert on host ("all-to-all dispatch"), padded to
    a common per-core count T_CORE, and each core runs its expert's FFN:
        hT = relu(W1e.T @ xT + b1e);  yT = W2e.T @ hT + b2e
    in bf16 on the tensor engine (fp32 PSUM accumulation).
  - Host applies the top-k combine weights and scatter-adds back ("combine").

Per-core device layout (everything 128-partition tiled):
  xt  [8,128,T]   bf16  x gathered+transposed, D on partitions (8 k-tiles)
  w1  [32,128,8,128] bf16  W1e chunk [f,p,k,:] = W1e[k*128+p, f*128:(f+1)*128]
  w2  [8,128,32,128] bf16  W2e chunk [d,p,f,:] = W2e[f*128+p, d*128:(d+1)*128]
  b1  [128,32] f32 (per-partition bias per f-chunk), b2 [128,8] f32
  yt  [8,128,T]   f32   output, D on partitions
"""

import numpy as np
import ml_dtypes

import concourse.bass as bass
import concourse.mybir as mybir
import concourse.tile as tile
from concourse import bacc
from concourse.bass_utils import run_bass_kernel_spmd

BF16 = mybir.dt.bfloat16
F32 = mybir.dt.float32

N_CORES = 8
P = 128

# Populated by kernel() with the BassKernelResults of the device run so a
# test harness can read exec_time_ns when tracing is enabled (BASS_TRACE=1).
LAST_RESULTS = None


def _build_moe_ffn(T_CORE: int, groups: list[tuple[int, int]], D: int, F: int):
    """One expert's FFN over T_CORE tokens: yT = W2.T @ relu(W1.T @ xT + b1) + b2."""
    KD = D // P   # k-tiles over D (contraction of layer 1)
    KF = F // P   # f-chunks over F (rows of hT / contraction of layer 2)
    ND = D // P   # d-chunks of the output

    nc = bacc.Bacc("TRN2", target_bir_lowering=False, debug=False,
                   num_devices=N_CORES)
    xt_d = nc.dram_tensor("xt", [KD, P, T_CORE], BF16, kind="ExternalInput")
    w1_d = nc.dram_tensor("w1", [KF, P, KD, P], BF16, kind="ExternalInput")
    w2_d = nc.dram_tensor("w2", [ND, P, KF, P], BF16, kind="ExternalInput")
    b1_d = nc.dram_tensor("b1", [P, KF], F32, kind="ExternalInput")
    b2_d = nc.dram_tensor("b2", [P, ND], F32, kind="ExternalInput")
    yt_d = nc.dram_tensor("yt", [ND, P, T_CORE], F32, kind="ExternalOutput")

    with tile.TileContext(nc) as tc:
        with (
            tc.tile_pool(name="resident", bufs=1) as rpool,
            tc.tile_pool(name="w1s", bufs=3) as w1pool,
            tc.tile_pool(name="w2s", bufs=2) as w2pool,
            tc.tile_pool(name="yout", bufs=3) as ypool,
            tc.tile_pool(name="ph", bufs=3, space="PSUM") as php,
            tc.tile_pool(name="py", bufs=3, space="PSUM") as pyp,
        ):
            xt_sb = rpool.tile([P, KD, T_CORE], BF16, tag="xt")
            for k in range(KD):
                nc.sync.dma_start(xt_sb[:, k, :], xt_d[k])
            b1_sb = rpool.tile([P, KF], F32, tag="b1")
            nc.sync.dma_start(b1_sb[:], b1_d[:])
            b2_sb = rpool.tile([P, ND], F32, tag="b2")
            nc.sync.dma_start(b2_sb[:], b2_d[:])
            h_sb = rpool.tile([P, KF, T_CORE], BF16, tag="h")

            # layer 1: hT[f] = relu(sum_k W1[k,f].T @ xT[k] + b1[f])
            for f in range(KF):
                w1f = w1pool.tile([P, KD, P], BF16, tag="w1f")
                nc.sync.dma_start(w1f[:], w1_d[f])
                for (g0, gn) in groups:
                    ph = php.tile([P, 512], F32, tag="ph", name="ph")[:, :gn]
                    for k in range(KD):
                        nc.tensor.matmul(
                            ph, w1f[:, k, :], xt_sb[:, k, g0:g0 + gn],
                            start=(k == 0), stop=(k == KD - 1),
                        )
                    nc.scalar.activation(
                        h_sb[:, f, g0:g0 + gn], ph,
                        mybir.ActivationFunctionType.Relu,
                        bias=b1_sb[:, f:f + 1],
                    )

            # layer 2: yT[d] = sum_f W2[f,d].T @ hT[f] + b2[d]
            for d in range(ND):
                w2d = w2pool.tile([P, KF, P], BF16, tag="w2d")
                nc.sync.dma_start(w2d[:], w2_d[d])
                for (g0, gn) in groups:
                    py = pyp.tile([P, 512], F32, tag="py", name="py")[:, :gn]
                    for f in range(KF):
                        nc.tensor.matmul(
                            py, w2d[:, f, :], h_sb[:, f, g0:g0 + gn],
                            start=(f == 0), stop=(f == KF - 1),
                        )
                    ysb = ypool.tile([P, 512], F32, tag="ysb", name="ysb")[:, :gn]
                    nc.vector.tensor_scalar_add(ysb, py, b2_sb[:, d:d + 1])
                    nc.sync.dma_start(yt_d[d][:, g0:g0 + gn], ysb)

    nc.compile()
    return nc


def kernel(x, Wr, br, W1, b1, W2, b2, top_k):
    x = np.asarray(x, dtype=np.float32)
    Wr = np.asarray(Wr, dtype=np.float32)
    br = np.asarray(br, dtype=np.float32)
    W1 = np.asarray(W1, dtype=np.float32)
    b1 = np.asarray(b1, dtype=np.float32)
    W2 = np.asarray(W2, dtype=np.float32)
    b2 = np.asarray(b2, dtype=np.float32)
    K = int(np.asarray(top_k))

    B, S, D = x.shape
    E = Wr.shape[1]
    F = W1.shape[2]
    T = B * S
    xt = x.reshape(T, D)

    # --- host router (replicated): f32 probs to match the reference, f64 top-k
    logits = xt @ Wr + br
    lmax = logits.max(axis=1, keepdims=True)
    pexp = np.exp(logits - lmax)
    probs = pexp / pexp.sum(axis=1, keepdims=True)          # [T, E] f32
    logits64 = xt.astype(np.float64) @ Wr.astype(np.float64) + br
    # top-k by descending prob, ties -> lower index (jax.lax.top_k semantics)
    topi = np.argsort(-logits64, axis=1, kind="stable")[:, :K]  # [T, K]

    # --- dispatch: token lists per expert
    tok_idx = [np.where((topi == e).any(axis=1))[0] for e in range(E)]
    counts = np.array([len(ix) for ix in tok_idx])
    T_CORE = max(P, int(np.ceil(counts.max() / P)) * P)
    groups = []
    off = 0
    while off < T_CORE:
        gn = min(512, T_CORE - off)
        groups.append((off, gn))
        off += gn

    in_maps = []
    for e in range(E):
        ix = tok_idx[e]
        xe = np.zeros((T_CORE, D), dtype=np.float32)
        xe[: len(ix)] = xt[ix]
        xte = np.ascontiguousarray(xe.T).astype(ml_dtypes.bfloat16)
        w1e = np.ascontiguousarray(
            W1[e].reshape(D // P, P, F // P, P).transpose(2, 1, 0, 3)
        ).astype(ml_dtypes.bfloat16)
        w2e = np.ascontiguousarray(
            W2[e].reshape(F // P, P, D // P, P).transpose(2, 1, 0, 3)
        ).astype(ml_dtypes.bfloat16)
        b1e = np.ascontiguousarray(b1[e].reshape(F // P, P).T)
        b2e = np.ascontiguousarray(b2[e].reshape(D // P, P).T)
        in_maps.append({
            "xt": xte.reshape(D // P, P, T_CORE),
            "w1": w1e,
            "w2": w2e,
            "b1": b1e,
            "b2": b2e,
        })

    nc = _build_moe_ffn(T_CORE, groups, D, F)
    res = run_bass_kernel_spmd(nc, in_maps, core_ids=list(range(N_CORES)))
    global LAST_RESULTS
    LAST_RESULTS = res

    # --- combine: out[t] += probs[t, e] * y_e[slot(t)]
    out = np.zeros((T, D), dtype=np.float32)
    for e in range(E):
        ix = tok_idx[e]
        yte = res.results[e]["yt"]                     # [D//P, P, T_CORE]
        ye = yte.reshape(D, T_CORE)[:, : len(ix)].T    # [c_e, D] token rows
        out[ix] += probs[ix, e][:, None] * ye

    return out.reshape(B, S, D), probs.reshape(B, S, E)
